# revision 11
# baseline (speedup 1.0000x reference)
"""GCN (DGL GraphConv norm='both', 5 stacked layers) on 8 Trainium2 NeuronCores.

Strategy (1D node partition, per the usual GNN sharding):
  - Nodes sharded contiguously across 8 cores (6250 nodes/core); edges
    partitioned by dst core. The small [5,128,128] weights are replicated.
  - Per layer, each core needs src rows from everywhere, so the scaled
    features hs = h * deg_out^-1/2 (stored fp16) are AllGathered into a
    per-core full [50000,128] HBM replica, then each core pulls its edges'
    rows with dma_gather (256B rows).
  - segment_sum over dst is a PE matmul against host-precomputed one-hot
    fp8 selection matrices (graph is static, so they are built once and
    streamed each layer): psumT[feat,dst] += rows_chunk^T @ Sel_chunk.
  - Dense part: h = relu(deg_in^-1/2 * (agg @ W) + b) with the bias folded
    into the PSUM accumulation as an outer product inv_nd (x) b, and both
    norms folded into the relu's per-partition scale.

Gather-path tuning (measured on-device with a pipelined-launch slope bench):
  - dma_gather per-row cost is latency-bound, not bandwidth-bound; deep
    pipelining of small pieces is what recovers throughput. 512-idx pieces
    across 4 SWDGE queues with a 16-buffer piece pool measured 1.3 ns/row
    vs 9 ns/row for 1024-idx pieces on 2 queues at depth 2. Exceeding the
    descriptor-ring capacity (e.g. 32 pieces in flight) collapses the rate
    again (Q7 spins in await_space), so the piece pool is the throttle.
  - Gathering from a Shared-address-space DRAM tensor is ~3.6x slower per
    row than from plain local DRAM, so each AllGather output is first
    copied (sequential DMA) into a local replica and gathered from there.

Host-side preprocessing only touches the static graph structure (degrees,
edge ordering, index/selection tables); all per-layer tensor math runs on
device.
"""

import hashlib

import numpy as np

import concourse.bass as bass
import concourse.mybir as mybir
import concourse.tile as tile
from concourse import bacc
from concourse.bass_utils import run_bass_kernel_spmd

N = 50000
E = 800000
D = 128
L = 5
NCORES = 8
NPC = N // NCORES          # 6250 nodes per core
TP = 128                   # tile partition (dst nodes per tile)
NT = (NPC + TP - 1) // TP  # 49 dst tiles per core (last has 106 rows)
LAST_ROWS = NPC - TP * (NT - 1)
# Node shard is split into part A (tiles 0..24, 3200 rows/core) and part B
# (tiles 25..48, 3050 real rows/core padded to 3072 so both parts are
# 128-divisible). Each part has its own AllGather buffer, so A's collective
# overlaps the tail of the layer and B's overlaps the next layer's A-side
# gathers. Row ids within a part also fit int16 (<= 25599).
ATILES = 25
ASZ = ATILES * TP            # 3200 rows per core in part A
BSZ = NT * TP - ASZ          # 3072 rows per core in part B (incl. 22 pad)
GROUPS = [list(range(g * 7, min(g * 7 + 7, NT))) for g in range(7)]

F32 = mybir.dt.float32
F16 = mybir.dt.float16
F8 = mybir.dt.float8e4
I16 = mybir.dt.int16

RG = [list(range(NCORES))]

# last kernel run's profiled exec time (filled by test harness runs w/ trace)
LAST_EXEC_NS = None

# debug-only: subsystems to skip when building the program (timing attribution)
DEBUG_SKIP = set()

# gather straight from the Shared AG output (skips the local-replica copy;
# slower per gather row but frees ~142us/layer of SDMA time)
GATHER_FROM_SHARED = False

# dma_gather piece tuning. HW-measured scaling (isolated, cc+sel skipped):
#   per-piece fixed ~1.0us on the Q7/SWDGE path, marginal ~0.32ns/row, so
#   bigger pieces win: 512-row pieces = 2.2ns/row, 1024 = 1.6ns/row.
#   >=2048-row pieces crash the device (descriptor ring overrun), and more
#   than ~4-5 entries in flight per queue collapses throughput (24 bufs of
#   512 = 6/queue measured 0.5ms SLOWER than 16 bufs = 4/queue).
GBUFS = 16
GPIECE = 1024  # rows per dma_gather piece

_CACHE = {}


def _cdiv(a, b):
    return -(-a // b)


def _make_schedule(src, dst):
    """Bake the (core-independent) chunk schedule from the edge lists."""
    core = dst // NPC
    loc = dst % NPC
    t = loc // TP
    dl = loc % TP
    v = ((src % NPC) >= ASZ).astype(np.int64)
    key = (core * NT + t) * 2 + v
    cnt = np.bincount(key, minlength=NCORES * NT * 2).reshape(NCORES, NT, 2)
    ch = _cdiv(cnt, 128)
    CH = ch.max(axis=0)  # [NT, 2] chunks per (tile, half), same for all cores

    chunk_off = np.zeros((NT, 2), np.int64)  # chunk offset inside (group,half) stream
    idxcol = np.zeros((7, 2), np.int64)      # idx col offset per (group,half)
    Kgv = np.zeros((7, 2), np.int64)         # num_idxs per (group,half)
    totch = 0
    idxcols = 0
    selbase = np.zeros((NT, 2), np.int64)
    for g, tl in enumerate(GROUPS):
        for vv in (0, 1):
            idxcol[g, vv] = idxcols
            off = 0
            for tt in tl:
                chunk_off[tt, vv] = off
                selbase[tt, vv] = totch
                off += int(CH[tt, vv])
                totch += int(CH[tt, vv])
            Kgv[g, vv] = off * 128
            idxcols += off * 8  # off*128/16 int16 cols
    return dict(
        CH=CH, chunk_off=chunk_off, selbase=selbase, idxcol=idxcol, Kgv=Kgv,
        TOTCH=totch, IDXCOLS=idxcols,
        core=core, t=t, dl=dl, v=v, key=key,
    )


def _make_core_inputs(sched, feat, src, dst, W, b):
    import ml_dtypes

    CH, chunk_off, selbase, idxcol = (
        sched["CH"], sched["chunk_off"], sched["selbase"], sched["idxcol"])
    TOTCH, IDXCOLS = sched["TOTCH"], sched["IDXCOLS"]
    key = sched["key"]

    deg_out = np.maximum(np.bincount(src, minlength=N), 1.0)
    deg_in = np.maximum(np.bincount(dst, minlength=N), 1.0)
    ns = (deg_out ** -0.5).astype(np.float32)
    nd = (deg_in ** -0.5).astype(np.float32)
    inv_nd = (1.0 / nd).astype(np.float32)

    order = np.lexsort((src, key))  # src-sorted within each segment
    sk = key[order]
    ssrc = src[order]
    sdl = sched["dl"][order]
    # rank of each edge within its (core,tile,half) segment
    seg_first = np.zeros(E, np.int64)
    newseg = np.r_[True, sk[1:] != sk[:-1]]
    seg_idx = np.cumsum(newseg) - 1
    firsts = np.flatnonzero(newseg)
    seg_first = firsts[seg_idx]
    rank = np.arange(E) - seg_first

    scc = sk // (NT * 2)
    rem = sk % (NT * 2)
    stt = rem // 2
    svv = rem % 2
    chl = rank // 128
    p = rank % 128
    gci = selbase[stt, svv] + chl
    stream_chunk = chunk_off[stt, svv] + chl
    gg = stt // 7
    i = stream_chunk * 128 + p
    col = idxcol[gg, svv] + i // 16
    row = i % 16
    soff = ssrc % NPC
    idxval = np.where(
        svv == 0,
        (ssrc // NPC) * ASZ + soff,
        (ssrc // NPC) * BSZ + (soff - ASZ),
    ).astype(np.int16)
    selcol = gci * 128 + sdl

    w_all = np.ascontiguousarray(
        np.concatenate([W[l] for l in range(L)], axis=1), dtype=np.float32
    )  # [128, 640] (fin, l*fout)
    b_all = np.ascontiguousarray(b[:L].reshape(1, L * D), dtype=np.float32)

    per_core = []
    for c in range(NCORES):
        m = scc == c
        idx_arr = np.zeros((16, IDXCOLS), np.int16)
        idx_arr[row[m], col[m]] = idxval[m]
        idx_arr = np.tile(idx_arr, (8, 1))  # replicated per Q7 core stripe
        sel_arr = np.zeros((128, TOTCH * 128), ml_dtypes.float8_e4m3)
        sel_arr[p[m], selcol[m]] = 1.0

        lo = c * NPC
        pad = NT * TP - NPC
        ndp = np.pad(nd[lo:lo + NPC], (0, pad)).reshape(NT, TP).T.copy()
        ndns = np.pad((nd * ns)[lo:lo + NPC], (0, pad)).reshape(NT, TP).T.copy()
        invndp = np.pad(inv_nd[lo:lo + NPC], (0, pad)).reshape(1, NT * TP).copy()

        # Prologue operands in partition-blocked layout: part A (3200 rows)
        # as [128, 3200] (node n at partition n//25), part B (3072 rows incl
        # pad) as [128, 3072], concatenated on the free dim. A flat traversal
        # of each part equals node-major row order, so one wide DMA per part
        # writes the bounce buffers exactly like the old per-tile path.
        featp_full = np.pad(feat[lo:lo + NPC].astype(np.float32),
                            ((0, pad), (0, 0)))
        nsflat = np.repeat(np.pad(ns[lo:lo + NPC], (0, pad)), D)
        featp = np.concatenate([
            featp_full[:ASZ].reshape(128, ASZ),
            featp_full[ASZ:].reshape(128, BSZ),
        ], axis=1)
        nsp = np.concatenate([
            nsflat[:ASZ * D].reshape(128, ASZ),
            nsflat[ASZ * D:].reshape(128, BSZ),
        ], axis=1)

        per_core.append({
            "featp": np.ascontiguousarray(featp, dtype=np.float32),
            "nsp": np.ascontiguousarray(nsp, dtype=np.float32),
            "idx": idx_arr,
            "sel": sel_arr,
            "w": w_all,
            "bb": b_all,
            "sc_mid": np.ascontiguousarray(ndns, dtype=np.float32),
            "sc_last": np.ascontiguousarray(ndp, dtype=np.float32),
            "invnd": invndp,
        })
    return per_core


def _build_program(sched):
    CH, chunk_off, idxcol, Kgv = (
        sched["CH"], sched["chunk_off"], sched["idxcol"], sched["Kgv"])
    selbase = sched["selbase"]
    TOTCH, IDXCOLS = sched["TOTCH"], sched["IDXCOLS"]

    nc = bacc.Bacc("TRN2", target_bir_lowering=False, debug=False, num_devices=NCORES,
                   num_swdge_queues=4)
    feat_in = nc.declare_dram_parameter("featp", [128, NT * TP], F32, isOutput=False)
    nsp_in = nc.declare_dram_parameter("nsp", [128, NT * TP], F32, isOutput=False)
    idx_in = nc.declare_dram_parameter("idx", [128, IDXCOLS], I16, isOutput=False)
    sel_in = nc.declare_dram_parameter("sel", [128, TOTCH * 128], F8, isOutput=False)
    w_in = nc.declare_dram_parameter("w", [D, L * D], F32, isOutput=False)
    b_in = nc.declare_dram_parameter("bb", [1, L * D], F32, isOutput=False)
    scmid_in = nc.declare_dram_parameter("sc_mid", [TP, NT], F32, isOutput=False)
    sclast_in = nc.declare_dram_parameter("sc_last", [TP, NT], F32, isOutput=False)
    invnd_in = nc.declare_dram_parameter("invnd", [1, NT * TP], F32, isOutput=False)
    out_ext = nc.declare_dram_parameter("out", [NPC, D], F32, isOutput=True)

    Relu = mybir.ActivationFunctionType.Relu

    with tile.TileContext(nc) as tc:
        with (
            tc.tile_pool(name="dramp", bufs=1, space="DRAM") as dp,
            tc.tile_pool(name="const", bufs=1) as cp,
            tc.tile_pool(name="gatp", bufs=GBUFS) as gpool,
            tc.tile_pool(name="workp", bufs=12) as wpool,
            tc.tile_pool(name="aggp", bufs=2) as apool,
            tc.tile_pool(name="psA", bufs=2, space="PSUM") as pA,
            tc.tile_pool(name="psB", bufs=2, space="PSUM") as pB,
        ):
            # Shared DRAM tensors allow a single writer inst -> one per
            # (layer, part). Part A covers tiles 0..ATILES-1, part B the rest.
            hsA = [
                dp.tile([NCORES * ASZ, D], F16, addr_space="Shared",
                        name=f"hsA{i}", bufs=1)
                for i in range(L)
            ]
            hsB = [
                dp.tile([NCORES * BSZ, D], F16, addr_space="Shared",
                        name=f"hsB{i}", bufs=1)
                for i in range(L)
            ]
            # Local (non-Shared) replicas of the gathered tables: dma_gather
            # from the Shared space runs ~3.6x slower per row, so each AG
            # output is first copied into plain DRAM and gathered from there.
            hsAl = [dp.tile([NCORES * ASZ, D], F16, name=f"hsAl{i}", bufs=1)
                    for i in (0, 1)]
            hsBl = [dp.tile([NCORES * BSZ, D], F16, name=f"hsBl{i}", bufs=1)
                    for i in (0, 1)]
            bnA = [dp.tile([ASZ, D], F16, name=f"bounceA{i}", bufs=1) for i in (0, 1)]
            bnB = [dp.tile([BSZ, D], F16, name=f"bounceB{i}", bufs=1) for i in (0, 1)]

            idx_sb = cp.tile([128, IDXCOLS], I16)
            nc.sync.dma_start(out=idx_sb[:, :], in_=idx_in[:, :])
            w_sb = cp.tile([D, L * D], F32)
            nc.sync.dma_start(out=w_sb[:, :], in_=w_in[:, :])
            b_sb = cp.tile([1, L * D], F32)
            nc.sync.dma_start(out=b_sb[:, :], in_=b_in[:, :])
            scmid_sb = cp.tile([TP, NT], F32)
            nc.sync.dma_start(out=scmid_sb[:, :], in_=scmid_in[:, :])
            sclast_sb = cp.tile([TP, NT], F32)
            nc.sync.dma_start(out=sclast_sb[:, :], in_=sclast_in[:, :])
            invnd_sb = cp.tile([1, NT * TP], F32)
            nc.sync.dma_start(out=invnd_sb[:, :], in_=invnd_in[:, :])

            # one gpsimd register per distinct gather length (dma_gather's
            # num_idxs_reg); to_reg inside the loop would exhaust the pool
            GCAP = GPIECE  # idxs per dma_gather piece (small pieces pipeline
                           # much deeper across the SWDGE rings)
            NQ = 4
            qctr = [0]   # round-robin gathers across the SWDGE queue rings
            kreg = {}
            for g in range(len(GROUPS)):
                for v in (0, 1):
                    K = int(Kgv[g, v])
                    while K > 0:
                        piece = min(K, GCAP)
                        if piece not in kreg:
                            kreg[piece] = nc.gpsimd.to_reg(piece)
                        K -= piece

            def rows_of(t):
                return TP if t < NT - 1 else LAST_ROWS

            def bounce_out(t, r, src_ap, which):
                # write rows of tile t into the right part bounce buffer
                if t < ATILES:
                    nc.sync.dma_start(
                        out=bnA[which][t * TP:t * TP + r, :], in_=src_ap)
                else:
                    b0 = (t - ATILES) * TP
                    nc.sync.dma_start(
                        out=bnB[which][b0:b0 + r, :], in_=src_ap)

            def emit_cc(part, which, lnext):
                buf = (hsA if part == 0 else hsB)[lnext]
                bn = (bnA if part == 0 else bnB)[which]
                nc.gpsimd.collective_compute(
                    "AllGather", mybir.AluOpType.bypass, replica_groups=RG,
                    ins=[bn.opt()], outs=[buf.opt()],
                )
                if not GATHER_FROM_SHARED:
                    loc = (hsAl if part == 0 else hsBl)[lnext % 2]
                    # split the Shared->local copy across both HWDGE queues:
                    # the Shared read path caps ~90GB/s per transfer, and this
                    # copy gates the next layer's gathers at each boundary
                    half = (NCORES * (ASZ if part == 0 else BSZ)) // 2
                    nc.sync.dma_start(out=loc[0:half, :], in_=buf[0:half, :])
                    nc.scalar.dma_start(out=loc[half:, :], in_=buf[half:, :])

            # ---- prologue: hs0 = feat * ns as one wide load + one DVE
            # multiply + one bounce DMA per part (partition-blocked layout;
            # flat order per part equals node-major row order). Tiles live in
            # a transient pool released before the resident sel pool opens so
            # the sel tables can take the space.
            Mul = mybir.AluOpType.mult
            with tc.tile_pool(name="prolog", bufs=1) as pp:
                ftw = pp.tile([128, NT * TP], F32)
                nsp_sb = pp.tile([128, NT * TP], F32)
                nc.sync.dma_start(out=nsp_sb[:, :], in_=nsp_in[:, :])
                h0w = pp.tile([128, NT * TP], F16)
                nc.sync.dma_start(out=ftw[:, 0:ASZ], in_=feat_in[:, 0:ASZ])
                nc.vector.scalar_tensor_tensor(
                    h0w[:, 0:ASZ], ftw[:, 0:ASZ], 1.0, nsp_sb[:, 0:ASZ], Mul, Mul)
                nc.sync.dma_start(out=bnA[0][:, :], in_=h0w[:, 0:ASZ])
                emit_cc(0, 0, 0)
                nc.sync.dma_start(out=ftw[:, ASZ:], in_=feat_in[:, ASZ:])
                nc.vector.scalar_tensor_tensor(
                    h0w[:, ASZ:], ftw[:, ASZ:], 1.0, nsp_sb[:, ASZ:], Mul, Mul)
                nc.sync.dma_start(out=bnB[0][:, :], in_=h0w[:, ASZ:])
                emit_cc(1, 0, 0)

            # Resident sel tables: one SBUF tile per group (both halves are
            # contiguous in the host sel layout), loaded once during layer 0
            # and reused by layers 1..4. 872 chunks x 128B/part ~= 109KB/part.
            spool = tc.alloc_tile_pool(name="selp", bufs=1)
            sel_sb = {}
            for g, tl in enumerate(GROUPS):
                gch = int(CH[tl[0]:tl[-1] + 1, :].sum())
                sel_sb[g] = spool.tile([128, gch * 128], F8, tag=f"sel{g}",
                                       name=f"sel{g}")

            def phase_b_group(tl, psTg, l):
                # one PSUM->SBUF copy per 7-tile group, then per-tile dense
                # matmuls into a shared 2-bank ps2 tile, per-tile relu + DMA
                GS = len(tl)
                aggT = apool.tile([D, GS * TP], F32, tag="aggT")
                nc.vector.tensor_copy(out=aggT[:, :], in_=psTg[:, :])
                ps2g = pB.tile([TP, GS * D], F32, tag="ps2")
                for ti, t in enumerate(tl):
                    nc.tensor.matmul(
                        ps2g[:, ti * D:(ti + 1) * D],
                        aggT[:, ti * TP:(ti + 1) * TP],
                        w_sb[:, l * D:(l + 1) * D],
                        start=True, stop=False,
                    )
                    nc.tensor.matmul(
                        ps2g[:, ti * D:(ti + 1) * D],
                        invnd_sb[0:1, t * TP:(t + 1) * TP],
                        b_sb[0:1, l * D:(l + 1) * D],
                        start=False, stop=True,
                    )
                for ti, t in enumerate(tl):
                    r = rows_of(t)
                    if l < L - 1:
                        hn = wpool.tile([TP, D], F16, tag="hsn")
                        nc.scalar.activation(
                            hn[0:r, :], ps2g[0:r, ti * D:(ti + 1) * D], Relu,
                            scale=scmid_sb[0:r, t:t + 1],
                        )
                        bounce_out(t, r, hn[0:r, :], (l + 1) % 2)
                        if "cc" not in DEBUG_SKIP:
                            if t == ATILES - 1:
                                emit_cc(0, (l + 1) % 2, l + 1)
                            elif t == NT - 1:
                                emit_cc(1, (l + 1) % 2, l + 1)
                    else:
                        hf = wpool.tile([TP, D], F32, tag="hfin")
                        nc.scalar.activation(
                            hf[0:r, :], ps2g[0:r, ti * D:(ti + 1) * D], Relu,
                            scale=sclast_sb[0:r, t:t + 1],
                        )
                        nc.sync.dma_start(
                            out=out_ext[t * TP:t * TP + r, :], in_=hf[0:r, :]
                        )

            # ---- layers
            PBLAG = 1  # groups of lag before phase_b_group, so the PE stream
                       # always has ready agg matmuls ahead of a dense mm that
                       # waits on the DVE copy round trip
            for l in range(L):
                li = 0 if "cc" in DEBUG_SKIP else l
                if GATHER_FROM_SHARED:
                    hs_parts = (hsA[li], hsB[li])
                else:
                    hs_parts = (hsAl[li % 2], hsBl[li % 2])
                pending = []
                CPP = GCAP // 128  # chunks per gather piece
                for g, tl in enumerate(GROUPS):
                    gts = {}
                    if l == 0 and "sel" not in DEBUG_SKIP:
                        # one resident-table fill per group, layer 0 only
                        gch = int(CH[tl[0]:tl[-1] + 1, :].sum())
                        sb0 = int(selbase[tl[0], 0])
                        nc.scalar.dma_start(
                            out=sel_sb[g][:, :],
                            in_=sel_in[:, sb0 * 128:(sb0 + gch) * 128],
                        )
                    CHG0 = int(Kgv[g, 0]) // 128  # v=1 column base in sel_sb[g]
                    for v in (0, 1):
                        K = int(Kgv[g, v])
                        if K == 0:
                            continue
                        ptiles = []
                        icol = int(idxcol[g, v])
                        done = 0
                        while done < K:
                            piece = min(K - done, GCAP)
                            if "gather" in DEBUG_SKIP:
                                ptiles.append(None)
                            else:
                                pt = gpool.tile([128, CPP, D], F16, tag="gat")
                                nc.gpsimd.dma_gather(
                                    pt[:, 0:piece // 128, :],
                                    hs_parts[v][:, :],
                                    idx_sb[:, icol + done // 16:icol + (done + piece) // 16],
                                    piece, kreg[piece], D,
                                    queue_num=qctr[0] % NQ,
                                )
                                qctr[0] += 1
                                ptiles.append(pt)
                            done += piece
                        gts[v] = ptiles
                    GS = len(tl)
                    psTg = pA.tile([D, GS * TP], F32, tag="psT")
                    for ti, t in enumerate(tl):
                        nch = int(CH[t, 0] + CH[t, 1])
                        psT = psTg[:, ti * TP:(ti + 1) * TP]
                        ci = 0
                        if ("aggmm" in DEBUG_SKIP or "gather" in DEBUG_SKIP
                                or "sel" in DEBUG_SKIP):
                            nc.tensor.matmul(
                                psT, w_sb[:, 0:TP], w_sb[:, 0:TP],
                                start=True, stop=True,
                            )
                            ci = nch
                        for v in (0, 1):
                            if ci >= nch:
                                break
                            for j in range(int(CH[t, v])):
                                sc = int(chunk_off[t, v]) + j
                                scol = sc + (CHG0 if v == 1 else 0)
                                nc.tensor.matmul(
                                    psT,
                                    gts[v][sc // CPP][:, sc % CPP, :],
                                    sel_sb[g][:, scol * 128:(scol + 1) * 128],
                                    start=(ci == 0), stop=(ci == nch - 1),
                                )
                                ci += 1
                                if ci >= nch:
                                    break
                    pending.append((tl, psTg, l))
                    if len(pending) > PBLAG:
                        phase_b_group(*pending.pop(0))
                for args in pending:
                    phase_b_group(*args)
            spool.release()
    nc.compile()
    return nc


def _get_compiled(src, dst):
    dig = hashlib.sha256(src.tobytes() + dst.tobytes()).hexdigest()
    if dig not in _CACHE:
        sched = _make_schedule(src, dst)
        nc = _build_program(sched)
        _CACHE[dig] = (sched, nc)
    return _CACHE[dig]


def kernel(feat, src, dst, W, b, trace=False):
    global LAST_EXEC_NS
    feat = np.asarray(feat, dtype=np.float32)
    src = np.asarray(src).astype(np.int64)
    dst = np.asarray(dst).astype(np.int64)
    W = np.asarray(W, dtype=np.float32)
    b = np.asarray(b, dtype=np.float32)

    sched, nc = _get_compiled(src, dst)
    in_maps = _make_core_inputs(sched, feat, src, dst, W, b)
    res = run_bass_kernel_spmd(nc, in_maps, list(range(NCORES)), trace=trace)
    LAST_EXEC_NS = res.exec_time_ns
    out = np.concatenate([res.results[c]["out"] for c in range(NCORES)], axis=0)
    return out.astype(np.float32)



# revision 18
# speedup vs baseline: 1.0025x; 1.0025x over previous
"""GCN (DGL GraphConv norm='both', 5 stacked layers) on 8 Trainium2 NeuronCores.

Strategy (1D node partition, per the usual GNN sharding):
  - Nodes sharded contiguously across 8 cores (6250 nodes/core); edges
    partitioned by dst core. The small [5,128,128] weights are replicated.
  - Per layer, each core needs src rows from everywhere, so the scaled
    features hs = h * deg_out^-1/2 (stored fp16) are AllGathered into a
    per-core full [50000,128] HBM replica, then each core pulls its edges'
    rows with dma_gather (256B rows).
  - segment_sum over dst is a PE matmul against host-precomputed one-hot
    fp8 selection matrices (graph is static, so they are built once and
    streamed each layer): psumT[feat,dst] += rows_chunk^T @ Sel_chunk.
  - Dense part: h = relu(deg_in^-1/2 * (agg @ W) + b) with the bias folded
    into the PSUM accumulation as an outer product inv_nd (x) b, and both
    norms folded into the relu's per-partition scale.

Gather-path tuning (measured on-device with a pipelined-launch slope bench):
  - dma_gather per-row cost is latency-bound, not bandwidth-bound; deep
    pipelining of small pieces is what recovers throughput. 512-idx pieces
    across 4 SWDGE queues with a 16-buffer piece pool measured 1.3 ns/row
    vs 9 ns/row for 1024-idx pieces on 2 queues at depth 2. Exceeding the
    descriptor-ring capacity (e.g. 32 pieces in flight) collapses the rate
    again (Q7 spins in await_space), so the piece pool is the throttle.
  - Gathering from a Shared-address-space DRAM tensor is ~3.6x slower per
    row than from plain local DRAM, so each AllGather output is first
    copied (sequential DMA) into a local replica and gathered from there.

Host-side preprocessing only touches the static graph structure (degrees,
edge ordering, index/selection tables); all per-layer tensor math runs on
device.
"""

import hashlib

import numpy as np

import concourse.bass as bass
import concourse.mybir as mybir
import concourse.tile as tile
from concourse import bacc
from concourse.bass_utils import run_bass_kernel_spmd

N = 50000
E = 800000
D = 128
L = 5
NCORES = 8
NPC = N // NCORES          # 6250 nodes per core
TP = 128                   # tile partition (dst nodes per tile)
NT = (NPC + TP - 1) // TP  # 49 dst tiles per core (last has 106 rows)
LAST_ROWS = NPC - TP * (NT - 1)
# Node shard is split into part A (tiles 0..24, 3200 rows/core) and part B
# (tiles 25..48, 3050 real rows/core padded to 3072 so both parts are
# 128-divisible). Each part has its own AllGather buffer, so A's collective
# overlaps the tail of the layer and B's overlaps the next layer's A-side
# gathers. Row ids within a part also fit int16 (<= 25599).
ATILES = 25
ASZ = ATILES * TP            # 3200 rows per core in part A
BSZ = NT * TP - ASZ          # 3072 rows per core in part B (incl. 22 pad)
GROUPS = [list(range(g * 7, min(g * 7 + 7, NT))) for g in range(7)]
PSTRIPS = 4                  # prologue strip count (per part)

F32 = mybir.dt.float32
F16 = mybir.dt.float16
F8 = mybir.dt.float8e4
I16 = mybir.dt.int16

RG = [list(range(NCORES))]

# last kernel run's profiled exec time (filled by test harness runs w/ trace)
LAST_EXEC_NS = None

# debug-only: subsystems to skip when building the program (timing attribution)
DEBUG_SKIP = set()

# gather straight from the Shared AG output (skips the local-replica copy;
# slower per gather row but frees ~142us/layer of SDMA time)
GATHER_FROM_SHARED = False

# AllGather directly into the Local-address-space gather tables (no Shared
# output buffer, no Shared->local copy). Non-shared-output collectives take
# a slower NRT path; measure before adopting.
AG_LOCAL = False

# dma_gather piece tuning. HW-measured scaling (isolated, cc+sel skipped):
#   per-piece fixed ~1.0us on the Q7/SWDGE path, marginal ~0.32ns/row, so
#   bigger pieces win: 512-row pieces = 2.2ns/row, 1024 = 1.6ns/row.
#   >=2048-row pieces crash the device (descriptor ring overrun), and more
#   than ~4-5 entries in flight per queue collapses throughput (24 bufs of
#   512 = 6/queue measured 0.5ms SLOWER than 16 bufs = 4/queue).
GBUFS = 16
GPIECE = 1024  # rows per dma_gather piece

_CACHE = {}


def _cdiv(a, b):
    return -(-a // b)


def _make_schedule(src, dst):
    """Bake the (core-independent) chunk schedule from the edge lists."""
    core = dst // NPC
    loc = dst % NPC
    t = loc // TP
    dl = loc % TP
    v = ((src % NPC) >= ASZ).astype(np.int64)
    key = (core * NT + t) * 2 + v
    cnt = np.bincount(key, minlength=NCORES * NT * 2).reshape(NCORES, NT, 2)
    ch = _cdiv(cnt, 128)
    CH = ch.max(axis=0)  # [NT, 2] chunks per (tile, half), same for all cores

    chunk_off = np.zeros((NT, 2), np.int64)  # chunk offset inside (group,half) stream
    idxcol = np.zeros((7, 2), np.int64)      # idx col offset per (group,half)
    Kgv = np.zeros((7, 2), np.int64)         # num_idxs per (group,half)
    totch = 0
    idxcols = 0
    selbase = np.zeros((NT, 2), np.int64)
    for g, tl in enumerate(GROUPS):
        for vv in (0, 1):
            idxcol[g, vv] = idxcols
            off = 0
            for tt in tl:
                chunk_off[tt, vv] = off
                selbase[tt, vv] = totch
                off += int(CH[tt, vv])
                totch += int(CH[tt, vv])
            Kgv[g, vv] = off * 128
            idxcols += off * 8  # off*128/16 int16 cols
    return dict(
        CH=CH, chunk_off=chunk_off, selbase=selbase, idxcol=idxcol, Kgv=Kgv,
        TOTCH=totch, IDXCOLS=idxcols,
        core=core, t=t, dl=dl, v=v, key=key,
    )


def _make_core_inputs(sched, feat, src, dst, W, b):
    import ml_dtypes

    CH, chunk_off, selbase, idxcol = (
        sched["CH"], sched["chunk_off"], sched["selbase"], sched["idxcol"])
    TOTCH, IDXCOLS = sched["TOTCH"], sched["IDXCOLS"]
    key = sched["key"]

    deg_out = np.maximum(np.bincount(src, minlength=N), 1.0)
    deg_in = np.maximum(np.bincount(dst, minlength=N), 1.0)
    ns = (deg_out ** -0.5).astype(np.float32)
    nd = (deg_in ** -0.5).astype(np.float32)
    inv_nd = (1.0 / nd).astype(np.float32)

    order = np.lexsort((src, key))  # src-sorted within each segment
    sk = key[order]
    ssrc = src[order]
    sdl = sched["dl"][order]
    # rank of each edge within its (core,tile,half) segment
    seg_first = np.zeros(E, np.int64)
    newseg = np.r_[True, sk[1:] != sk[:-1]]
    seg_idx = np.cumsum(newseg) - 1
    firsts = np.flatnonzero(newseg)
    seg_first = firsts[seg_idx]
    rank = np.arange(E) - seg_first

    scc = sk // (NT * 2)
    rem = sk % (NT * 2)
    stt = rem // 2
    svv = rem % 2
    chl = rank // 128
    p = rank % 128
    gci = selbase[stt, svv] + chl
    stream_chunk = chunk_off[stt, svv] + chl
    gg = stt // 7
    i = stream_chunk * 128 + p
    col = idxcol[gg, svv] + i // 16
    row = i % 16
    soff = ssrc % NPC
    idxval = np.where(
        svv == 0,
        (ssrc // NPC) * ASZ + soff,
        (ssrc // NPC) * BSZ + (soff - ASZ),
    ).astype(np.int16)
    selcol = gci * 128 + sdl

    w_all = np.ascontiguousarray(
        np.concatenate([W[l] for l in range(L)], axis=1), dtype=np.float32
    )  # [128, 640] (fin, l*fout)
    b_all = np.ascontiguousarray(b[:L].reshape(1, L * D), dtype=np.float32)

    # Replicated full-graph feature/norm tables in layer-0-table layout:
    # A-part rows of every core concatenated, then B-part rows (padded),
    # strip-blocked so each strip's [128, R] flat order equals row order.
    pad = NT * TP - NPC
    AF = np.concatenate([feat[c * NPC:c * NPC + ASZ] for c in range(NCORES)], 0)
    BF = np.concatenate(
        [np.pad(feat[c * NPC + ASZ:(c + 1) * NPC], ((0, pad), (0, 0)))
         for c in range(NCORES)], 0)
    nsA = np.concatenate([ns[c * NPC:c * NPC + ASZ] for c in range(NCORES)])
    nsB = np.concatenate(
        [np.pad(ns[c * NPC + ASZ:(c + 1) * NPC], (0, pad)) for c in range(NCORES)])
    AST = NCORES * ASZ // PSTRIPS
    BST = NCORES * BSZ // PSTRIPS
    featp_rep = np.ascontiguousarray(np.concatenate(
        [AF[s * AST:(s + 1) * AST].reshape(128, AST) for s in range(PSTRIPS)]
        + [BF[s * BST:(s + 1) * BST].reshape(128, BST) for s in range(PSTRIPS)],
        axis=1), dtype=np.float16)
    nsA_rows = np.repeat(nsA, D).reshape(-1, D)
    nsB_rows = np.repeat(nsB, D).reshape(-1, D)
    nsp_rep = np.ascontiguousarray(np.concatenate(
        [nsA_rows[s * AST:(s + 1) * AST].reshape(128, AST) for s in range(PSTRIPS)]
        + [nsB_rows[s * BST:(s + 1) * BST].reshape(128, BST) for s in range(PSTRIPS)],
        axis=1), dtype=np.float16)

    per_core = []
    for c in range(NCORES):
        m = scc == c
        idx_arr = np.zeros((16, IDXCOLS), np.int16)
        idx_arr[row[m], col[m]] = idxval[m]
        idx_arr = np.tile(idx_arr, (8, 1))  # replicated per Q7 core stripe
        sel_arr = np.zeros((128, TOTCH * 128), ml_dtypes.float8_e4m3)
        sel_arr[p[m], selcol[m]] = 1.0

        lo = c * NPC
        pad = NT * TP - NPC
        ndp = np.pad(nd[lo:lo + NPC], (0, pad)).reshape(NT, TP).T.copy()
        ndns = np.pad((nd * ns)[lo:lo + NPC], (0, pad)).reshape(NT, TP).T.copy()
        invndp = np.pad(inv_nd[lo:lo + NPC], (0, pad)).reshape(1, NT * TP).copy()

        per_core.append({
            "featp": featp_rep,
            "nsp": nsp_rep,
            "idx": idx_arr,
            "sel": sel_arr,
            "w": w_all,
            "bb": b_all,
            "sc_mid": np.ascontiguousarray(ndns, dtype=np.float32),
            "sc_last": np.ascontiguousarray(ndp, dtype=np.float32),
            "invnd": invndp,
        })
    return per_core


def _build_program(sched):
    CH, chunk_off, idxcol, Kgv = (
        sched["CH"], sched["chunk_off"], sched["idxcol"], sched["Kgv"])
    selbase = sched["selbase"]
    TOTCH, IDXCOLS = sched["TOTCH"], sched["IDXCOLS"]

    nc = bacc.Bacc("TRN2", target_bir_lowering=False, debug=False, num_devices=NCORES,
                   num_swdge_queues=4)
    FULL = NCORES * (ASZ + BSZ)
    feat_in = nc.declare_dram_parameter("featp", [128, FULL], F16, isOutput=False)
    nsp_in = nc.declare_dram_parameter("nsp", [128, FULL], F16, isOutput=False)
    idx_in = nc.declare_dram_parameter("idx", [128, IDXCOLS], I16, isOutput=False)
    sel_in = nc.declare_dram_parameter("sel", [128, TOTCH * 128], F8, isOutput=False)
    w_in = nc.declare_dram_parameter("w", [D, L * D], F32, isOutput=False)
    b_in = nc.declare_dram_parameter("bb", [1, L * D], F32, isOutput=False)
    scmid_in = nc.declare_dram_parameter("sc_mid", [TP, NT], F32, isOutput=False)
    sclast_in = nc.declare_dram_parameter("sc_last", [TP, NT], F32, isOutput=False)
    invnd_in = nc.declare_dram_parameter("invnd", [1, NT * TP], F32, isOutput=False)
    out_ext = nc.declare_dram_parameter("out", [NPC, D], F32, isOutput=True)

    Relu = mybir.ActivationFunctionType.Relu

    with tile.TileContext(nc) as tc:
        with (
            tc.tile_pool(name="dramp", bufs=1, space="DRAM") as dp,
            tc.tile_pool(name="const", bufs=1) as cp,
            tc.tile_pool(name="gatp", bufs=GBUFS) as gpool,
            tc.tile_pool(name="workp", bufs=12) as wpool,
            tc.tile_pool(name="aggp", bufs=2) as apool,
            tc.tile_pool(name="psA", bufs=2, space="PSUM") as pA,
            tc.tile_pool(name="psB", bufs=2, space="PSUM") as pB,
        ):
            # Shared DRAM tensors allow a single writer inst -> one per
            # (layer, part). Part A covers tiles 0..ATILES-1, part B the rest.
            hsA = [
                dp.tile([NCORES * ASZ, D], F16, addr_space="Shared",
                        name=f"hsA{i}", bufs=1)
                for i in range(L)
            ]
            hsB = [
                dp.tile([NCORES * BSZ, D], F16, addr_space="Shared",
                        name=f"hsB{i}", bufs=1)
                for i in range(L)
            ]
            # Local (non-Shared) replicas of the gathered tables: dma_gather
            # from the Shared space runs ~3.6x slower per row, so each AG
            # output is first copied into plain DRAM and gathered from there.
            hsAl = [dp.tile([NCORES * ASZ, D], F16, name=f"hsAl{i}", bufs=1)
                    for i in (0, 1)]
            hsBl = [dp.tile([NCORES * BSZ, D], F16, name=f"hsBl{i}", bufs=1)
                    for i in (0, 1)]
            bnA = [dp.tile([ASZ, D], F16, name=f"bounceA{i}", bufs=1) for i in (0, 1)]
            bnB = [dp.tile([BSZ, D], F16, name=f"bounceB{i}", bufs=1) for i in (0, 1)]

            idx_sb = cp.tile([128, IDXCOLS], I16)
            nc.sync.dma_start(out=idx_sb[:, :], in_=idx_in[:, :])
            w_sb = cp.tile([D, L * D], F32)
            nc.sync.dma_start(out=w_sb[:, :], in_=w_in[:, :])
            b_sb = cp.tile([1, L * D], F32)
            nc.sync.dma_start(out=b_sb[:, :], in_=b_in[:, :])
            scmid_sb = cp.tile([TP, NT], F32)
            nc.sync.dma_start(out=scmid_sb[:, :], in_=scmid_in[:, :])
            sclast_sb = cp.tile([TP, NT], F32)
            nc.sync.dma_start(out=sclast_sb[:, :], in_=sclast_in[:, :])
            invnd_sb = cp.tile([1, NT * TP], F32)
            nc.sync.dma_start(out=invnd_sb[:, :], in_=invnd_in[:, :])

            # one gpsimd register per distinct gather length (dma_gather's
            # num_idxs_reg); to_reg inside the loop would exhaust the pool
            GCAP = GPIECE  # idxs per dma_gather piece (small pieces pipeline
                           # much deeper across the SWDGE rings)
            NQ = 4
            qctr = [0]   # round-robin gathers across the SWDGE queue rings
            kreg = {}
            for g in range(len(GROUPS)):
                for v in (0, 1):
                    K = int(Kgv[g, v])
                    while K > 0:
                        piece = min(K, GCAP)
                        if piece not in kreg:
                            kreg[piece] = nc.gpsimd.to_reg(piece)
                        K -= piece

            def rows_of(t):
                return TP if t < NT - 1 else LAST_ROWS

            def bounce_out(t, r, src_ap, which):
                # write rows of tile t into the right part bounce buffer
                if t < ATILES:
                    nc.sync.dma_start(
                        out=bnA[which][t * TP:t * TP + r, :], in_=src_ap)
                else:
                    b0 = (t - ATILES) * TP
                    nc.sync.dma_start(
                        out=bnB[which][b0:b0 + r, :], in_=src_ap)

            def emit_cc(part, which, lnext):
                bn = (bnA if part == 0 else bnB)[which]
                if AG_LOCAL:
                    loc = (hsAl if part == 0 else hsBl)[lnext % 2]
                    nc.gpsimd.collective_compute(
                        "AllGather", mybir.AluOpType.bypass, replica_groups=RG,
                        ins=[bn.opt()], outs=[loc.opt()],
                    )
                    return
                buf = (hsA if part == 0 else hsB)[lnext]
                nc.gpsimd.collective_compute(
                    "AllGather", mybir.AluOpType.bypass, replica_groups=RG,
                    ins=[bn.opt()], outs=[buf.opt()],
                )
                if not GATHER_FROM_SHARED:
                    loc = (hsAl if part == 0 else hsBl)[lnext % 2]
                    # split the Shared->local copy across both HWDGE queues:
                    # the Shared read path caps ~90GB/s per transfer, and this
                    # copy gates the next layer's gathers at each boundary
                    half = (NCORES * (ASZ if part == 0 else BSZ)) // 2
                    nc.sync.dma_start(out=loc[0:half, :], in_=buf[0:half, :])
                    nc.scalar.dma_start(out=loc[half:, :], in_=buf[half:, :])

            # ---- prologue: every core receives the FULL feat (f16,
            # strip-blocked) and builds the layer-0 tables locally with one
            # DVE multiply per strip -- no prologue collectives at all.
            # Strip s of part A is feat rows [s*AST, (s+1)*AST) of the
            # concatenated-across-cores A-part, stored as [128, AST] whose
            # flat order equals node-row-major order.
            Mul = mybir.AluOpType.mult
            AST = NCORES * ASZ // PSTRIPS
            BST = NCORES * BSZ // PSTRIPS
            h0A = hsA[0] if GATHER_FROM_SHARED else hsAl[0]
            h0B = hsB[0] if GATHER_FROM_SHARED else hsBl[0]
            with tc.tile_pool(name="prolog", bufs=2) as pp:
                for s in range(PSTRIPS):
                    ft = pp.tile([128, AST], F16, tag="pft", name="pft")
                    nc.sync.dma_start(
                        out=ft[:, :], in_=feat_in[:, s * AST:(s + 1) * AST])
                    nst = pp.tile([128, AST], F16, tag="pns", name="pns")
                    nc.sync.dma_start(
                        out=nst[:, :], in_=nsp_in[:, s * AST:(s + 1) * AST])
                    h0 = pp.tile([128, AST], F16, tag="ph0", name="ph0")
                    nc.vector.scalar_tensor_tensor(
                        h0[:, :], ft[:, :], 1.0, nst[:, :], Mul, Mul)
                    nc.scalar.dma_start(
                        out=h0A[s * AST:(s + 1) * AST, :], in_=h0[:, :])
                for s in range(PSTRIPS):
                    a0 = NCORES * ASZ
                    ft = pp.tile([128, AST], F16, tag="pft", name="pft")
                    nc.sync.dma_start(
                        out=ft[:, 0:BST], in_=feat_in[:, a0 + s * BST:a0 + (s + 1) * BST])
                    nst = pp.tile([128, AST], F16, tag="pns", name="pns")
                    nc.sync.dma_start(
                        out=nst[:, 0:BST], in_=nsp_in[:, a0 + s * BST:a0 + (s + 1) * BST])
                    h0 = pp.tile([128, AST], F16, tag="ph0", name="ph0")
                    nc.vector.scalar_tensor_tensor(
                        h0[:, 0:BST], ft[:, 0:BST], 1.0, nst[:, 0:BST], Mul, Mul)
                    nc.scalar.dma_start(
                        out=h0B[s * BST:(s + 1) * BST, :], in_=h0[:, 0:BST])

            # Resident sel tables: one SBUF tile per group (both halves are
            # contiguous in the host sel layout), loaded once during layer 0
            # and reused by layers 1..4. 872 chunks x 128B/part ~= 109KB/part.
            spool = tc.alloc_tile_pool(name="selp", bufs=1)
            sel_sb = {}
            for g, tl in enumerate(GROUPS):
                gch = int(CH[tl[0]:tl[-1] + 1, :].sum())
                sel_sb[g] = spool.tile([128, gch * 128], F8, tag=f"sel{g}",
                                       name=f"sel{g}")

            def phase_b_group(tl, psTg, l):
                # one PSUM->SBUF copy per 7-tile group, then per-tile dense
                # matmuls into a shared 2-bank ps2 tile, per-tile relu + DMA
                GS = len(tl)
                aggT = apool.tile([D, GS * TP], F32, tag="aggT")
                nc.vector.tensor_copy(out=aggT[:, :], in_=psTg[:, :])
                ps2g = pB.tile([TP, GS * D], F32, tag="ps2")
                for ti, t in enumerate(tl):
                    nc.tensor.matmul(
                        ps2g[:, ti * D:(ti + 1) * D],
                        aggT[:, ti * TP:(ti + 1) * TP],
                        w_sb[:, l * D:(l + 1) * D],
                        start=True, stop=False,
                    )
                    nc.tensor.matmul(
                        ps2g[:, ti * D:(ti + 1) * D],
                        invnd_sb[0:1, t * TP:(t + 1) * TP],
                        b_sb[0:1, l * D:(l + 1) * D],
                        start=False, stop=True,
                    )
                for ti, t in enumerate(tl):
                    r = rows_of(t)
                    if l < L - 1:
                        hn = wpool.tile([TP, D], F16, tag="hsn")
                        nc.scalar.activation(
                            hn[0:r, :], ps2g[0:r, ti * D:(ti + 1) * D], Relu,
                            scale=scmid_sb[0:r, t:t + 1],
                        )
                        bounce_out(t, r, hn[0:r, :], (l + 1) % 2)
                        if "cc" not in DEBUG_SKIP:
                            if t == ATILES - 1:
                                emit_cc(0, (l + 1) % 2, l + 1)
                            elif t == NT - 1:
                                emit_cc(1, (l + 1) % 2, l + 1)
                    else:
                        hf = wpool.tile([TP, D], F32, tag="hfin")
                        nc.scalar.activation(
                            hf[0:r, :], ps2g[0:r, ti * D:(ti + 1) * D], Relu,
                            scale=sclast_sb[0:r, t:t + 1],
                        )
                        nc.sync.dma_start(
                            out=out_ext[t * TP:t * TP + r, :], in_=hf[0:r, :]
                        )

            # ---- layers
            PBLAG = 1  # groups of lag before phase_b_group, so the PE stream
                       # always has ready agg matmuls ahead of a dense mm that
                       # waits on the DVE copy round trip
            for l in range(L):
                li = 0 if "cc" in DEBUG_SKIP else l
                if GATHER_FROM_SHARED:
                    hs_parts = (hsA[li], hsB[li])
                else:
                    hs_parts = (hsAl[li % 2], hsBl[li % 2])
                pending = []
                CPP = GCAP // 128  # chunks per gather piece
                for g, tl in enumerate(GROUPS):
                    gts = {}
                    if l == 0 and "sel" not in DEBUG_SKIP:
                        # one resident-table fill per group, layer 0 only
                        gch = int(CH[tl[0]:tl[-1] + 1, :].sum())
                        sb0 = int(selbase[tl[0], 0])
                        nc.scalar.dma_start(
                            out=sel_sb[g][:, :],
                            in_=sel_in[:, sb0 * 128:(sb0 + gch) * 128],
                        )
                    CHG0 = int(Kgv[g, 0]) // 128  # v=1 column base in sel_sb[g]
                    for v in (0, 1):
                        K = int(Kgv[g, v])
                        if K == 0:
                            continue
                        ptiles = []
                        icol = int(idxcol[g, v])
                        done = 0
                        while done < K:
                            piece = min(K - done, GCAP)
                            if "gather" in DEBUG_SKIP:
                                ptiles.append(None)
                            else:
                                pt = gpool.tile([128, CPP, D], F16, tag="gat")
                                nc.gpsimd.dma_gather(
                                    pt[:, 0:piece // 128, :],
                                    hs_parts[v][:, :],
                                    idx_sb[:, icol + done // 16:icol + (done + piece) // 16],
                                    piece, kreg[piece], D,
                                    queue_num=qctr[0] % NQ,
                                )
                                qctr[0] += 1
                                ptiles.append(pt)
                            done += piece
                        gts[v] = ptiles
                    GS = len(tl)
                    psTg = pA.tile([D, GS * TP], F32, tag="psT")
                    for ti, t in enumerate(tl):
                        nch = int(CH[t, 0] + CH[t, 1])
                        psT = psTg[:, ti * TP:(ti + 1) * TP]
                        ci = 0
                        if ("aggmm" in DEBUG_SKIP or "gather" in DEBUG_SKIP
                                or "sel" in DEBUG_SKIP):
                            nc.tensor.matmul(
                                psT, w_sb[:, 0:TP], w_sb[:, 0:TP],
                                start=True, stop=True,
                            )
                            ci = nch
                        for v in (0, 1):
                            if ci >= nch:
                                break
                            for j in range(int(CH[t, v])):
                                sc = int(chunk_off[t, v]) + j
                                scol = sc + (CHG0 if v == 1 else 0)
                                nc.tensor.matmul(
                                    psT,
                                    gts[v][sc // CPP][:, sc % CPP, :],
                                    sel_sb[g][:, scol * 128:(scol + 1) * 128],
                                    start=(ci == 0), stop=(ci == nch - 1),
                                )
                                ci += 1
                                if ci >= nch:
                                    break
                    pending.append((tl, psTg, l))
                    if len(pending) > PBLAG:
                        phase_b_group(*pending.pop(0))
                for args in pending:
                    phase_b_group(*args)
            spool.release()
    nc.compile()
    return nc


def _get_compiled(src, dst):
    dig = hashlib.sha256(src.tobytes() + dst.tobytes()).hexdigest()
    if dig not in _CACHE:
        sched = _make_schedule(src, dst)
        nc = _build_program(sched)
        _CACHE[dig] = (sched, nc)
    return _CACHE[dig]


def kernel(feat, src, dst, W, b, trace=False):
    global LAST_EXEC_NS
    feat = np.asarray(feat, dtype=np.float32)
    src = np.asarray(src).astype(np.int64)
    dst = np.asarray(dst).astype(np.int64)
    W = np.asarray(W, dtype=np.float32)
    b = np.asarray(b, dtype=np.float32)

    sched, nc = _get_compiled(src, dst)
    in_maps = _make_core_inputs(sched, feat, src, dst, W, b)
    res = run_bass_kernel_spmd(nc, in_maps, list(range(NCORES)), trace=trace)
    LAST_EXEC_NS = res.exec_time_ns
    out = np.concatenate([res.results[c]["out"] for c in range(NCORES)], axis=0)
    return out.astype(np.float32)

